# revision 1
# baseline (speedup 1.0000x reference)
"""Trainium2 Bass kernel for a ViT-style transformer block (B=16, N=577,
H=768, NH=12, MLP=3072) with the reference's Q@V^T attention quirk.

Sharding: data-parallel over batch — 8 NeuronCores x 2 batch items each.
All weights replicated. On-chip dataflow is channel-major ("CT": channels
on partitions, tokens on the free axis); the host pre-transposes x and
back-transposes the output. x, weights and activations stream as bf16
(fp32 PSUM accumulation); residual/x1 kept bf16 with fp32 epilogues.

Host-side exact refactorings baked into the shipped weights:
 - LN gains/betas folded into Wq/Wv/W1 rows and the matching biases.
 - bv folded into bo via the softmax-rows-sum-to-1 identity.
 - negated column sums of wv shipped for the token-major v epilogue.

Schedule highlights:
 - token-major v (v_tm) is computed straight from x during the LN1
   latency chain (LN affine applied per-token afterwards via PE-transposed
   rstd/mu*rstd columns), so the PE is never idle at startup;
 - emission interleave: qv(b1) rides inside att(b0)'s head loop and
   wo(b0)/ln2(b0)/fc1(b0,j<12) inside att(b1)'s, with scores running two
   heads ahead of out_head; interleaved fc1 tiles write Identity+bias
   (shares the Exp ACT table) and gelu is applied in place later behind a
   data-dependency gate;
 - scores get their own 2-deep PSUM pool so exp-paced score tiles don't
   share a slot chain with the slower-freeing pnt/prr tiles; the pool is
   reused by the post-attention fc1/fc2 tiles so the pure-MLP phase
   double-buffers across four PSUM slots instead of chaining behind
   LN2(b1)'s slow-freeing stat tiles;
 - elementwise work is spread across DVE/Pool/ACT (squares + LN subtract
   on gpsimd, b1 projection epilogues on DVE, the rest on ACT), and
   weight/const DMAs ride the ACT hw-dge queue so the SP queue belongs
   to the x tiles.

Token axis padded to 580 per batch item; every post-matmul op consumes a
two-bank PSUM tile (halves at columns 0 and 512) in a single strided AP.

Self-contained: hardcodes shapes; only needs /opt/trn_rl_repo.
"""
import sys

sys.path.insert(0, "/opt/trn_rl_repo")

import numpy as np
import ml_dtypes

import concourse.bass as bass
import concourse.tile as tile
from concourse import bacc, mybir
from concourse import bass_utils

P = 128
B, T, H, NH, HD, MLP = 16, 577, 768, 12, 64, 3072
NCORES = 8
B2 = B // NCORES          # batch items per core
KT = H // P               # 6 hidden tiles
JT = MLP // P             # 24 mlp tiles
TP = 640                  # padded token columns for lhsT-windowed CT tensors
TE = 580                  # padded token columns (2 x 290, even halves)
HB = 290                  # token half-block
TT = [(0, 128), (128, 128), (256, 128), (384, 128), (512, 65)]  # token tiles
EPS = 1e-6

f32 = mybir.dt.float32
f32r = mybir.dt.float32r
bf16 = mybir.dt.bfloat16
AF = mybir.ActivationFunctionType
OP = mybir.AluOpType


def halves(pt, parts=P):
    """View a [*,1024] two-bank psum tile as [parts, 2, HB] (cols 0.. and 512..)."""
    return pt.rearrange("p (two c) -> p two c", two=2)[:parts, :, :HB]


def build_program(repeat=1, upto="fc2"):
    nc = bacc.Bacc("TRN2", target_bir_lowering=False, debug=False, num_devices=NCORES)

    xt_d = nc.dram_tensor("xt", [B2, H, T], bf16, kind="ExternalInput").ap()
    wq_d = nc.dram_tensor("wq", [KT, P, KT, P], bf16, kind="ExternalInput").ap()
    wv_d = nc.dram_tensor("wv", [H, H], bf16, kind="ExternalInput").ap()
    ncs_d = nc.dram_tensor("ncs", [P, H], bf16, kind="ExternalInput").ap()
    wo_d = nc.dram_tensor("wo", [KT, P, KT, P], bf16, kind="ExternalInput").ap()
    w1_d = nc.dram_tensor("w1", [JT, P, KT, P], bf16, kind="ExternalInput").ap()
    w2_d = nc.dram_tensor("w2", [KT, P, JT, P], bf16, kind="ExternalInput").ap()
    bq_d = nc.dram_tensor("bq", [H], f32, kind="ExternalInput").ap()
    bv_d = nc.dram_tensor("bv", [H], f32, kind="ExternalInput").ap()
    bo_d = nc.dram_tensor("bo", [H], f32, kind="ExternalInput").ap()
    b1_d = nc.dram_tensor("b1", [MLP], f32, kind="ExternalInput").ap()
    b2_d = nc.dram_tensor("b2", [H], f32, kind="ExternalInput").ap()
    out_d = nc.dram_tensor("outt", [B2, H, T], f32, kind="ExternalOutput").ap()

    with tile.TileContext(nc) as tc:
        with (
            tc.tile_pool(name="persist", bufs=1) as persist,
            tc.tile_pool(name="wstream", bufs=3) as wstream,
            tc.tile_pool(name="w2pool", bufs=2) as w2pool,
            tc.tile_pool(name="epool", bufs=4, space="SBUF") as epool,
            tc.tile_pool(name="scratch", bufs=2) as scratch,
            tc.tile_pool(name="rbpool", bufs=2) as rbpool,
            tc.tile_pool(name="rows", bufs=1) as rows,
            tc.tile_pool(name="outp", bufs=2) as outp,
            tc.tile_pool(name="consts", bufs=1) as consts,
        ):
            ones_k = consts.tile([P, 1], f32r, tag="ones_k")
            nc.vector.memset(ones_k.bitcast(f32), 1.0)
            ones_kb = consts.tile([P, 1], bf16, tag="ones_kb")
            nc.vector.memset(ones_kb, 1.0)
            ones_r = consts.tile([1, P], f32r, tag="ones_r")
            nc.vector.memset(ones_r.bitcast(f32), 1.0)
            eps_t = consts.tile([1, 1], f32, tag="eps_t")
            nc.vector.memset(eps_t, EPS)
            id2 = consts.tile([1, 1], f32, tag="id2")
            nc.vector.memset(id2, 1.0)

            const_dmas = []

            def col_load(name, dram, ntiles):
                t = consts.tile([P, ntiles], f32, tag=name)
                # DMA issued inside emit_block, after wv/ncs on the ACT queue
                const_dmas.append((t, dram.rearrange("(k p) -> p k", p=P)))
                return t

            bq_c = col_load("bq_c", bq_d, KT)
            bv_c = col_load("bv_c", bv_d, KT)
            bo_c = col_load("bo_c", bo_d, KT)
            b2_c = col_load("b2_c", b2_d, KT)
            b1_c = col_load("b1_c", b1_d, JT)

            for _ in range(repeat):
                emit_block(nc, tc, persist, wstream, w2pool, epool, scratch, rbpool,
                           rows, outp, ones_k, ones_kb, ones_r, eps_t, id2, const_dmas,
                           bq_c, bv_c, bo_c, b2_c, b1_c,
                           xt_d, wq_d, wv_d, ncs_d, wo_d, w1_d, w2_d, out_d, upto)
    nc.compile()
    return nc


def emit_block(nc, tc, persist, wstream, w2pool, epool, scratch, rbpool, rows, outp,
               ones_k, ones_kb, ones_r, eps_t, id2, const_dmas, bq_c, bv_c, bo_c, b2_c, b1_c,
               xt_d, wq_d, wv_d, ncs_d, wo_d, w1_d, w2_d, out_d, upto="fc2"):
    TU = B2 * TE
    cu = lambda b: b * TE
    ct = lambda b: b * TP

    # Persistent tensors. Tags encode slot reuse; each second tenant's first
    # write is ordered after the first tenant's last read on every engine.
    xTs = [persist.tile([P, KT, TE], bf16, tag="sC", name="xT0"),
           persist.tile([P, KT, TE], bf16, tag="sD", name="xT1")]
    yT = persist.tile([P, KT, B2 * TP], bf16, tag="sA")
    qT = persist.tile([P, KT, TU], bf16, tag="sE")
    vT = persist.tile([P, KT, B2 * TP], bf16, tag="sF")
    v_tm = persist.tile([P, B2, 5, NH * 65], bf16, tag="sG")
    wv_sb = persist.tile([P, KT, H], bf16, tag="sB")
    ncs_sb = persist.tile([P, H], bf16, tag="sN")
    rtms = [rows.tile([P, 5, 2], f32, tag="rtm0", name="rtm0"),
            rows.tile([P, 5, 2], f32, tag="rtm1", name="rtm1")]

    def f(ap):
        return ap.bitcast(f32)

    def two(ap):
        return ap.rearrange("p (two c) -> p two c", two=2)

    # ---- load x (zero the 3 pad token columns) and wv ----
    for b in range(B2):
        for kt in range(KT):
            nc.sync.dma_start(xTs[b][:, kt, :T], xt_d[b, kt * P:(kt + 1) * P, :])
            nc.gpsimd.memset(xTs[b][:, kt, T:TE], 0.0)
    nc.scalar.dma_start(wv_sb, wv_d.rearrange("(kt p) n -> p kt n", p=P))
    nc.scalar.dma_start(ncs_sb, ncs_d)
    for _t, _ap in const_dmas:
        nc.scalar.dma_start(_t, _ap)
    const_dmas.clear()
    for b in range(B2):
        for kt in range(KT):
            nc.gpsimd.memset(yT[:, kt, ct(b) + TE:ct(b) + TP], 0.0)
            nc.gpsimd.memset(vT[:, kt, ct(b) + TE:ct(b) + TP], 0.0)

    # ---- layernorm for one batch item (gains/betas folded into weights) ----
    # Engine split: Square on Pool (SBUF->SBUF), stats chain on DVE, the
    # rstd/mu*rstd broadcasts land in PSUM and are copied once to SBUF so the
    # per-tile scale runs as mult(DVE) + subtract(Pool), both SBUF-only.
    def layernorm_b(src, s0, dst, d0, ps):
        bfsrc = src.dtype == bf16
        sf = (lambda ap: ap) if bfsrc else f
        ones_s1 = ones_kb if bfsrc else ones_k
        sfull = lambda kt: src[:, kt, s0:s0 + TE]
        shalf = lambda kt, i: src[:, kt, s0 + i * HB:s0 + (i + 1) * HB]
        s1 = ps.tile([1, 1024], f32, tag="p", name="ln_s1")
        for kt in range(KT):
            for i in range(2):
                nc.tensor.matmul(s1[:, i * 512:i * 512 + HB], ones_s1, shalf(kt, i),
                                 start=kt == 0, stop=kt == KT - 1)
        s2 = ps.tile([1, 1024], f32, tag="p", name="ln_s2")
        for kt in range(KT):
            sq = scratch.tile([P, TE], bf16, tag="sq")
            nc.gpsimd.tensor_tensor(sq, sf(sfull(kt)), sf(sfull(kt)), OP.mult)
            for i in range(2):
                nc.tensor.matmul(s2[:, i * 512:i * 512 + HB], ones_kb,
                                 sq[:, i * HB:(i + 1) * HB],
                                 start=kt == 0, stop=kt == KT - 1)
        mu = rows.tile([1, TE], f32, tag="mu")
        nc.vector.tensor_scalar_mul(two(mu), halves(s1, 1), 1.0 / H)
        var = rows.tile([1, TE], f32, tag="var")
        nc.vector.tensor_tensor(var, mu, mu, OP.mult)
        nc.vector.scalar_tensor_tensor(two(var), halves(s2, 1), 1.0 / H,
                                       two(var), OP.mult, OP.subtract)
        nc.scalar.activation(var, var, AF.Sqrt, bias=eps_t)
        rstd = rows.tile([1, TE], f32r, tag="rstd")
        with nc.allow_low_precision(reason="f32r rounding of rstd"):
            nc.vector.reciprocal(rstd, var)
        murstd = rows.tile([1, TE], f32r, tag="var")
        nc.vector.tensor_tensor(murstd, mu, f(rstd), OP.mult)
        rb_r = ps.tile([P, 1024], f32, tag="p", name="ln_rbr")
        rb_mr = ps.tile([P, 1024], f32, tag="p", name="ln_rbmr")
        for i in range(2):
            nc.tensor.matmul(rb_r[:, i * 512:i * 512 + HB], ones_r,
                             rstd[:, i * HB:(i + 1) * HB], start=True, stop=True)
            nc.tensor.matmul(rb_mr[:, i * 512:i * 512 + HB], ones_r,
                             murstd[:, i * HB:(i + 1) * HB], start=True, stop=True)
        rb_r_sb = rbpool.tile([P, TE], bf16, tag="rbr", name="rb_r_sb")
        nc.vector.tensor_copy(two(rb_r_sb), halves(rb_r))
        rb_mr_sb = rbpool.tile([P, TE], bf16, tag="rbmr", name="rb_mr_sb")
        nc.scalar.activation(two(rb_mr_sb), halves(rb_mr), AF.Identity)
        for kt in range(KT):
            tmp = scratch.tile([P, TE], bf16, tag="lntmp")
            nc.vector.tensor_tensor(tmp, sf(sfull(kt)), rb_r_sb, OP.mult)
            nc.gpsimd.tensor_tensor(dst[:, kt, d0:d0 + TE], tmp,
                                    rb_mr_sb, OP.subtract)
        return rstd, murstd

    with (tc.tile_pool(name="ps_ln1", bufs=2, space="PSUM") as ps_ln,
          tc.tile_pool(name="ps_vtm", bufs=4, space="PSUM") as ps_vtm):
        # token-major v from xT directly (no LN dependency): the matmuls fill
        # the PE during the LN1 latency chain; epilogues apply the LN affine
        # per-token once the transposed stats arrive:
        #   v_tm[t,c] = r[t]*(Wv^T x)[t,c] + (-colsum(wv))[c]*(mu[t]*r[t])
        vtm_ps = {}

        def vtm_mms(b, ti, nb):
            t0, tl = TT[ti]
            w = min(TE - t0, P)
            pv = ps_vtm.tile([P, 512], f32, tag="pv", name="vtm_ps")
            for kt in range(KT):
                nc.tensor.matmul(
                    pv[:w, :384], xTs[b][:, kt, t0:t0 + w],
                    wv_sb[:, kt, nb * 384:(nb + 1) * 384],
                    start=kt == 0, stop=kt == KT - 1)
            vtm_ps[(b, ti, nb)] = pv

        def vtm_epi(b, ti, nb):
            rr = 128 if ti < 4 else 65
            pv = vtm_ps.pop((b, ti, nb))
            r_tm = rtms[b][:rr, ti, 0:1]
            m2_tm = rtms[b][:rr, ti, 1:2]
            tmp = scratch.tile([P, 384], bf16, tag="vtmp")
            nc.vector.tensor_scalar(tmp[:rr], pv[:rr, :384], r_tm, None, OP.mult)
            nc.vector.scalar_tensor_tensor(
                v_tm[:rr, b, ti, 65 * nb * 6: 65 * (nb + 1) * 6]
                    .rearrange("p (h d) -> p h d", d=65)[:, :, :HD],
                ncs_sb[:rr, nb * 384:(nb + 1) * 384]
                    .rearrange("p (h d) -> p h d", d=HD),
                m2_tm, tmp[:rr].rearrange("p (h d) -> p h d", d=HD),
                OP.mult, OP.add)

        def export_rtm(b, rmpair):
            rstd, murstd = rmpair
            for ti in range(5):
                t0, tl = TT[ti]
                w = min(TE - t0, P)
                ptr = ps_ln.tile([P, 1024], f32, tag="p", name="tr_ps")
                nc.tensor.transpose(ptr[:w, 0:1], f(rstd)[0:1, t0:t0 + w],
                                    id2)
                nc.tensor.transpose(ptr[:w, 1:2], f(murstd)[0:1, t0:t0 + w],
                                    id2)
                nc.vector.tensor_copy(rtms[b][:w, ti, :], ptr[:w, 0:2])

        for b in range(B2):
            nc.gpsimd.memset(v_tm[:, b, 4, :], 0.0)
            for ti in range(5):
                rr = 128 if ti < 4 else 65
                nc.gpsimd.memset(v_tm[:rr, b, ti, HD::65], 1.0)
        for ti in range(5):
            for nb in range(2):
                vtm_mms(0, ti, nb)
        rm0 = layernorm_b(xTs[0], 0, yT, ct(0), ps_ln)
        export_rtm(0, rm0)
        for ti in range(5):
            for nb in range(2):
                vtm_mms(1, ti, nb)
        rm1 = layernorm_b(xTs[1], 0, yT, ct(1), ps_ln)
        export_rtm(1, rm1)
        for b in range(B2):
            for ti in range(5):
                for nb in range(2):
                    vtm_epi(b, ti, nb)
    if upto == "ln1":
        return

    # ---- q/v, attention, wo+LN2, MLP — emitted with cross-phase interleave:
    # qv(b1) units ride inside att(b0)'s head loop, fc1(b0, j0-11) units ride
    # inside att(b1)'s. Emission order drives both the scheduler priorities
    # and the PSUM tag-"p" slot-reuse chain, so PE fill work is available
    # whenever the softmax (ACT) pipeline lags. ----
    oT = persist.tile([P, KT, TU], bf16, tag="sH")
    x1T = persist.tile([P, KT, TU], bf16, tag="sA")
    y2Ts = [persist.tile([P, KT, TE], bf16, tag="sB", name="y2T0"),
            persist.tile([P, KT, TE], bf16, tag="sC", name="y2T1")]
    mtbox = [None]  # fc1 output chunks; 2 fresh slots + 2 reused (sG/sH)

    with (tc.tile_pool(name="ps_mid", bufs=2, space="PSUM") as ps,
          tc.tile_pool(name="ps_sc", bufs=2, space="PSUM") as ps_sc):

        def q_unit(b, j):
            wqj = wstream.tile([P, KT, P], bf16, tag="w", name="wq_j")
            nc.sync.dma_start(wqj, wq_d[j])
            pq = ps.tile([P, 1024], f32, tag="p", name="q_ps")
            for kt in range(KT):
                for i in range(2):
                    nc.tensor.matmul(
                        pq[:, i * 512:i * 512 + HB], wqj[:, kt, :],
                        yT[:, kt, ct(b) + i * HB: ct(b) + (i + 1) * HB],
                        start=kt == 0, stop=kt == KT - 1)
            if b == 0:
                nc.scalar.activation(two(qT[:, j, cu(b):cu(b) + TE]),
                                     halves(pq), AF.Identity, bias=bq_c[:, j:j + 1])
            else:
                nc.vector.tensor_scalar(two(qT[:, j, cu(b):cu(b) + TE]),
                                        halves(pq), bq_c[:, j:j + 1], None, OP.add)

        def v_unit(b, j):
            pv = ps.tile([P, 1024], f32, tag="p", name="v_ps")
            for kt in range(KT):
                for i in range(2):
                    nc.tensor.matmul(
                        pv[:, i * 512:i * 512 + HB], wv_sb[:, kt, j * P:(j + 1) * P],
                        yT[:, kt, ct(b) + i * HB: ct(b) + (i + 1) * HB],
                        start=kt == 0, stop=kt == KT - 1)
            if b == 0:
                nc.scalar.activation(two(vT[:, j, ct(b):ct(b) + TE]),
                                     halves(pv), AF.Identity, bias=bv_c[:, j:j + 1])
            else:
                nc.vector.tensor_scalar(two(vT[:, j, ct(b):ct(b) + TE]),
                                        halves(pv), bv_c[:, j:j + 1], None, OP.add)

        def qv_units(b):
            us = [lambda j=j: q_unit(b, j) for j in range(KT)]
            us += [lambda j=j: v_unit(b, j) for j in range(KT)]
            return us

        def scores(b, h):
            jh, ph = h // 2, HD * (h % 2)
            E = epool.tile([P, 5, TE], bf16, tag="E")
            for kt in range(5):
                pss = ps_sc.tile([P, 1024], f32, tag="pss", name="s_ps")
                for i in range(2):
                    nc.tensor.matmul(
                        pss[:, i * 512:i * 512 + HB],
                        vT[ph:ph + HD, jh, ct(b) + kt * P: ct(b) + (kt + 1) * P],
                        qT[ph:ph + HD, jh, cu(b) + i * HB: cu(b) + (i + 1) * HB],
                        start=True, stop=True)
                nc.scalar.activation(two(E[:, kt, :]), halves(pss),
                                     AF.Exp, scale=0.125)
            return E

        def out_head(b, h, E):
            jh, ph = h // 2, HD * (h % 2)
            pnt = ps.tile([P, 1024], f32, tag="p", name="nt_ps")
            for kt in range(5):
                for i in range(2):
                    nc.tensor.matmul(pnt[:65, i * 512:i * 512 + HB],
                                     v_tm[:, b, kt, 65 * h: 65 * h + 65],
                                     E[:, kt, i * HB:(i + 1) * HB],
                                     start=kt == 0, stop=kt == 4)
            r = rows.tile([1, TE], f32r, tag="r")
            with nc.allow_low_precision(reason="f32r rounding of softmax denom"):
                nc.vector.reciprocal(two(r), halves(pnt, 128)[64:65])
            prr = ps_sc.tile([P, 1024], f32, tag="pss", name="rep_ps")
            for i in range(2):
                nc.tensor.matmul(prr[:HD, i * 512:i * 512 + HB], ones_r[:, :HD],
                                 r[:, i * HB:(i + 1) * HB], start=True, stop=True)
            ntsb = scratch.tile([P, TE], f32, tag="ntsb")
            nc.vector.tensor_copy(two(ntsb[:HD]), halves(pnt, HD))
            nc.vector.tensor_tensor(
                two(oT[ph:ph + HD, jh, cu(b):cu(b) + TE]),
                two(ntsb[:HD]), halves(prr, HD), OP.mult)

        def attention_b(b, fill, per_head):
            # scores run two heads ahead of out_head so the exp (ACT) chain
            # never waits behind pnt/prr matmuls in the PE queue
            Es = {}
            for h in range(NH):
                Es[h] = scores(b, h)
                if h >= 2:
                    out_head(b, h - 2, Es.pop(h - 2))
                for _ in range(per_head):
                    if fill:
                        fill.pop(0)()
            out_head(b, NH - 2, Es.pop(NH - 2))
            out_head(b, NH - 1, Es.pop(NH - 1))
            while fill:
                fill.pop(0)()

        def wo_unit(b, j):
            woj = wstream.tile([P, KT, P], bf16, tag="wbf", name="wo_j")
            nc.sync.dma_start(woj, wo_d[j])
            po = ps.tile([P, 1024], f32, tag="p", name="wo_ps")
            for kt in range(KT):
                for i in range(2):
                    nc.tensor.matmul(
                        po[:, i * 512:i * 512 + HB], woj[:, kt, :],
                        oT[:, kt, cu(b) + i * HB: cu(b) + (i + 1) * HB],
                        start=kt == 0, stop=kt == KT - 1)
            nc.vector.scalar_tensor_tensor(
                two(x1T[:, j, cu(b):cu(b) + TE]), halves(po), bo_c[:, j:j + 1],
                two(xTs[b][:, j, :]), OP.add, OP.add)

        def wo_ln2(b):
            for j in range(KT):
                wo_unit(b, j)
            layernorm_b(x1T, cu(b), y2Ts[b], 0, ps)

        def fc1_unit(bb, j, defer_gelu=False, pool=None, ptag="p"):
            w1j = wstream.tile([P, KT, P], bf16, tag="w", name="w1_j")
            nc.sync.dma_start(w1j, w1_d[j])
            pm = (pool or ps).tile([P, 1024], f32, tag=ptag, name="fc1_ps")
            for kt in range(KT):
                for i in range(2):
                    nc.tensor.matmul(
                        pm[:, i * 512:i * 512 + HB], w1j[:, kt, :],
                        y2Ts[bb][:, kt, i * HB:(i + 1) * HB],
                        start=kt == 0, stop=kt == KT - 1)
            dst = two(mtbox[0][j // KT][:, j % KT, cu(bb):cu(bb) + TE])
            if defer_gelu:
                # raw+bias on DVE (ACT is exp-saturated mid-attention);
                # gelu applied in place later, gated behind attention's end
                nc.vector.tensor_scalar(dst, halves(pm), b1_c[:, j:j + 1],
                                        None, OP.add)
            else:
                nc.scalar.activation(dst, halves(pm), AF.Gelu,
                                     bias=b1_c[:, j:j + 1])

        def fc2_chain(bb, j2, w2j):
            pf = (ps_sc if (j2 + bb) % 2 else ps).tile(
                [P, 1024], f32, tag="pss" if (j2 + bb) % 2 else "p",
                name="fc2_ps")
            for j in range(JT):
                for i in range(2):
                    nc.tensor.matmul(
                        pf[:, i * 512:i * 512 + HB], w2j[:, j, :],
                        mtbox[0][j // KT][:, j % KT, cu(bb) + i * HB: cu(bb) + (i + 1) * HB],
                        start=j == 0, stop=j == JT - 1)
            ob = outp.tile([P, TE], f32, tag="ob")
            nc.vector.scalar_tensor_tensor(
                two(ob), halves(pf), b2_c[:, j2:j2 + 1],
                two(x1T[:, j2, cu(bb):cu(bb) + TE]), OP.add, OP.add)
            nc.sync.dma_start(
                out_d[bb, j2 * P:(j2 + 1) * P, :], ob[:, :T])

        for u in qv_units(0):
            u()
        if upto == "qv":
            return
        attention_b(0, qv_units(1), 2)
        if upto in ("att", "wo", "ln2"):
            wo_ln2(0)
            return
        mtbox[0] = [persist.tile([P, KT, TU], bf16, tag=t, name=f"mt_{t}")
                    for t in ("sI", "sJ", "sG", "sH")]
        fill1 = [lambda j=j: wo_unit(0, j) for j in range(KT)]
        fill1.append(lambda: layernorm_b(x1T, cu(0), y2Ts[0], 0, ps))
        fill1 += [lambda j=j: fc1_unit(0, j, defer_gelu=True)
                  for j in range(2 * KT)]
        attention_b(1, fill1, 2)
        wo_ln2(1)
        # zero bias gate, data-dependent on the last out_head of att(b1):
        # keeps the deferred gelus (a different ACT table) from hoisting into
        # the attention window and thrashing LoadActFuncSet.
        gate0 = rows.tile([P, 1], f32, tag="gate")
        nc.gpsimd.tensor_scalar(gate0, f(oT.bitcast(f32)[:, KT - 1, (cu(1) + TE) // 2 - 1: (cu(1) + TE) // 2]),
                                0.0, None, OP.mult)
        for j in range(2 * KT):
            sl = mtbox[0][j // KT][:, j % KT, cu(0):cu(0) + TE]
            nc.scalar.activation(sl, sl, AF.Gelu, bias=gate0)
        for j in range(2 * KT, JT):
            fc1_unit(0, j, pool=ps_sc, ptag="pss")
        for j in range(JT):
            fc1_unit(1, j, pool=ps_sc, ptag="pss")
        if upto == "fc1":
            return
        for j2 in range(KT):
            w2j = w2pool.tile([P, JT, P], bf16, tag="w2", name="w2_j")
            nc.sync.dma_start(w2j, w2_d[j2])
            for bb in range(B2):
                fc2_chain(bb, j2, w2j)


_cached = {}


def get_program(repeat=1):
    if repeat not in _cached:
        _cached[repeat] = build_program(repeat)
    return _cached[repeat]


def make_in_maps(inputs):
    x = np.asarray(inputs["x"], dtype=np.float32)
    xt_all = np.ascontiguousarray(x.transpose(0, 2, 1))  # [B, H, T]
    g1 = np.asarray(inputs["ln1_g"], np.float64)
    be1 = np.asarray(inputs["ln1_b"], np.float64)
    g2 = np.asarray(inputs["ln2_g"], np.float64)
    be2 = np.asarray(inputs["ln2_b"], np.float64)
    Wq = np.asarray(inputs["Wq"], np.float64)
    Wv = np.asarray(inputs["Wv"], np.float64)
    Wo = np.asarray(inputs["Wo"], np.float64)
    W1 = np.asarray(inputs["W1"], np.float64)
    # Fold LN affine into the consuming projections (exact refactoring).
    wq = g1[:, None] * Wq
    bq = be1 @ Wq + np.asarray(inputs["bq"], np.float64)
    wv = g1[:, None] * Wv
    bv = be1 @ Wv + np.asarray(inputs["bv"], np.float64)
    # v_tm carries no bias; probs rows sum to 1 so P@(1 x bv) == bv -> fold into bo.
    bo = np.asarray(inputs["bo"], np.float64) + bv @ Wo
    w1 = g2[:, None] * W1
    b1 = be2 @ W1 + np.asarray(inputs["b1"], np.float64)
    def prep(w, jt, dt):
        # [H_in, J*128] -> [j, p, kt, n] so each per-j DMA is fully contiguous
        kt = w.shape[0] // P
        return np.ascontiguousarray(
            w.reshape(kt, P, jt, P).transpose(2, 1, 0, 3)).astype(dt)

    ncs = np.ascontiguousarray(
        np.broadcast_to(-(wv.sum(axis=0)), (P, H))).astype(ml_dtypes.bfloat16)
    com = {
        "wq": prep(wq, KT, ml_dtypes.bfloat16),
        "ncs": ncs,
        "wv": wv.astype(ml_dtypes.bfloat16),
        "wo": prep(np.asarray(inputs["Wo"], np.float64), KT, ml_dtypes.bfloat16),
        "w1": prep(w1, JT, ml_dtypes.bfloat16),
        "w2": prep(np.asarray(inputs["W2"], np.float64), KT, ml_dtypes.bfloat16),
        "bq": bq.astype(np.float32),
        "bv": bv.astype(np.float32),
        "bo": bo.astype(np.float32),
        "b1": b1.astype(np.float32),
        "b2": np.asarray(inputs["b2"], np.float32),
    }
    return [dict(com, xt=np.ascontiguousarray(xt_all[i * B2:(i + 1) * B2]).astype(ml_dtypes.bfloat16))
            for i in range(NCORES)]


def kernel(**inputs):
    nc = get_program()
    in_maps = make_in_maps(inputs)
    res = bass_utils.run_bass_kernel_spmd(nc, in_maps, core_ids=list(range(NCORES)))
    out = np.concatenate([res.results[i]["outt"] for i in range(NCORES)], axis=0)
    return np.ascontiguousarray(out.transpose(0, 2, 1)).astype(np.float32)

